# revision 24
# baseline (speedup 1.0000x reference)
"""Causal self-attention (B=2, T=2048, C=1024, H=16, D=64) on 8 trn2 cores.

Sharding: tensor-parallel over (batch, head-group). Core c handles batch
c//4 and heads 4*(c%4) .. 4*(c%4)+4. Each core computes its 4 heads'
QKV projection, causal attention, and the partial output projection
(W_proj row-shard). The 4 partials per batch are summed on the host
(equivalent to the Megatron all-reduce, done at gather time).

On-core dataflow is feature-major ("transposed") throughout:
  X^T (PE transpose) -> Q^T,K^T = W^T X^T ; V natural = X^T^T Wv
  S^T[k,q] = K Q^T per 128-k-chunk (causal: q >= k-chunk start)
  P^T = exp(S^T/8) (ACT), diag-block masked (DVE)
  O'^T[65,q] += V'[k,:]^T P^T  where V' carries a ones column, so row 64
  accumulates the softmax denominator. O^T = O'^T[0:64] * recip(row 64).
  Y^T = W_proj^T O^T + b_proj  -> [1024, 2048] partial per core.

All matmul operands are float32r (fp22 truncation, full PE rate at
free-dim >= 256).
"""
import os
import sys
import numpy as np

B, T, C = 2, 2048, 1024
H, D = 16, 64
HPC = 4                 # heads per core
QC = HPC * D            # 256 qkv cols per core
NCORES = 8
NT = T // 128           # 16 T-chunks of 128
NT4 = T // 512          # 4 T-chunks of 512
NKC = 8                 # contraction chunks over C
SCALE = 1.0 / np.sqrt(D)

_cache = {}


def _build():
    import concourse.bass as bass
    import concourse.bacc as bacc
    import concourse.mybir as mybir
    import concourse.tile as tile

    F32 = mybir.dt.float32
    F32R = mybir.dt.float32r
    F16 = mybir.dt.float16
    AF = mybir.ActivationFunctionType

    nc = bacc.Bacc()
    x_d = nc.dram_tensor("x", [T, C], F32, kind="ExternalInput")
    wq_d = nc.dram_tensor("wq", [C, QC], F16, kind="ExternalInput")
    wk_d = nc.dram_tensor("wk", [C, QC], F16, kind="ExternalInput")
    wv_d = nc.dram_tensor("wv", [C, QC], F16, kind="ExternalInput")
    bq_d = nc.dram_tensor("bq", [128, 2], F32, kind="ExternalInput")
    bk_d = nc.dram_tensor("bk", [128, 2], F32, kind="ExternalInput")
    bv_d = nc.dram_tensor("bv", [1, QC], F32, kind="ExternalInput")
    wp_d = nc.dram_tensor("wp", [QC, C], F16, kind="ExternalInput")
    bp_d = nc.dram_tensor("bp", [128, 8], F32, kind="ExternalInput")
    mask_d = nc.dram_tensor("mask", [128, 128], F16, kind="ExternalInput")
    id_d = nc.dram_tensor("ident", [128, 128], F32, kind="ExternalInput")
    on_d = nc.dram_tensor("ones4", [128, QC], F16, kind="ExternalInput")
    yt_d = nc.dram_tensor("yt", [C, T], F32, kind="ExternalOutput")

    with tile.TileContext(nc) as tc:
        with tc.tile_pool(name="cst", bufs=1) as cst, \
             tc.tile_pool(name="wgt", bufs=1) as wgt, \
             tc.tile_pool(name="xin", bufs=8) as xin, \
             tc.tile_pool(name="xt", bufs=16) as xtp, \
             tc.tile_pool(name="qk", bufs=1) as qkp, \
             tc.tile_pool(name="vv", bufs=1) as vvp, \
             tc.tile_pool(name="pp", bufs=6) as ppp, \
             tc.tile_pool(name="dn", bufs=1) as dnp, \
             tc.tile_pool(name="yy", bufs=2) as yyp, \
             tc.tile_pool(name="mm", bufs=2, space="PSUM") as mmp, \
             tc.tile_pool(name="ss", bufs=2, space="PSUM") as ssp, \
             tc.tile_pool(name="po", bufs=2, space="PSUM") as pop:

            # ---- constants / weights ----
            ident = cst.tile([128, 128], F32R, tag="id")
            nc.scalar.dma_start(out=ident[:], in_=id_d[:].bitcast(F32R))
            mask = cst.tile([128, 128], F16, tag="mask")
            nc.scalar.dma_start(out=mask[:], in_=mask_d[:])

            wq_s = wgt.tile([128, NKC * QC], F16, tag="wq")
            wk_s = wgt.tile([128, NKC * QC], F16, tag="wk")
            wv_s = wgt.tile([128, NKC * QC], F16, tag="wv")
            for wd, ws in ((wq_d, wq_s), (wk_d, wk_s), (wv_d, wv_s)):
                nc.scalar.dma_start(
                    out=ws[:].rearrange("p (c n) -> p c n", n=QC),
                    in_=wd.ap().rearrange("(c p) n -> p c n", p=128))
            wp_s = [wgt.tile([128, C], F16, tag=f"wp{k}", name=f"wp{k}") for k in range(2)]
            for k in range(2):
                nc.scalar.dma_start(out=wp_s[k][:],
                                  in_=wp_d[k * 128:(k + 1) * 128, :])
            bq_s = cst.tile([128, 2], F32, tag="bq")
            bk_s = cst.tile([128, 2], F32, tag="bk")
            bp_s = cst.tile([128, 8], F32, tag="bp")
            nc.scalar.dma_start(out=bq_s[:], in_=bq_d[:])
            nc.scalar.dma_start(out=bk_s[:], in_=bk_d[:])
            nc.scalar.dma_start(out=bp_s[:], in_=bp_d[:])
            bv_row = cst.tile([1, QC], F32, tag="bvr")
            nc.scalar.dma_start(out=bv_row[:], in_=bv_d[:])
            bv_b = cst.tile([128, QC], F32, tag="bvb")
            nc.gpsimd.partition_broadcast(bv_b[:], bv_row[:])

            # ---- persistent activations ----
            # Q^T / K^T: two tiles each, heads (2m, 2m+1) stacked on partitions
            qt_s = [qkp.tile([128, T], F16, tag=f"qt{m}", name=f"qt{m}") for m in range(2)]
            kt_s = [qkp.tile([128, T], F16, tag=f"kt{m}", name=f"kt{m}") for m in range(2)]
            # V' tiles: [128, 4*65], per head 64 v-cols + ones col
            vp_s = [vvp.tile([128, HPC * 2 * D], F16, tag=f"vp{i}",
                             name=f"vp{i}") for i in range(NT)]
            # O^T: heads (2m, 2m+1) stacked
            ot_s = [qkp.tile([128, T], F16, tag=f"ot{m}", name=f"ot{m}") for m in range(2)]

            # ---- fused rounds over 512-wide T-chunks ----
            # Round t4: QKV projections for T-chunk t4, then attention
            # q-chunk cq=t4 for all heads (kc = 0..4*t4+3 k-chunks are all
            # available), then the output-projection columns for chunk t4.
            # The Tile scheduler overlaps rounds by dataflow, keeping the
            # PE dense while ACT runs the exps.
            for t4 in range(NT4):
                xt_c = []
                for i in range(4):
                    xl = xin.tile([128, C], F32R, tag="x")
                    xt_c.append(xl)
                for half in range(2):
                    for i in range(4):
                        nc.sync.dma_start(
                            out=xt_c[i][:, half * 512:(half + 1) * 512],
                            in_=x_d[(4 * t4 + i) * 128:(4 * t4 + i + 1) * 128,
                                    half * 512:(half + 1) * 512]
                            .bitcast(F32R))
                xts = []
                for c in range(NKC):
                    pxt = mmp.tile([128, 512], F32R, tag="mm")
                    for i in range(4):
                        nc.tensor.transpose(pxt[:, i * 128:(i + 1) * 128],
                                            xt_c[i][:, c * 128:(c + 1) * 128],
                                            ident[:])
                    xs = xtp.tile([128, 512], F16, tag="xt")
                    nc.vector.tensor_copy(xs[:], pxt[:])
                    xts.append(xs)
                # Q^T, K^T chunks [128, 512]
                for (ws, bs, dst) in ((wq_s, bq_s, qt_s), (wk_s, bk_s, kt_s)):
                    for m in range(2):
                        pq = mmp.tile([128, 512], F32, tag="mm")
                        for c in range(NKC):
                            nc.tensor.matmul(
                                pq[:],
                                ws[:, c * QC + m * 128:c * QC + (m + 1) * 128],
                                xts[c][:],
                                start=(c == 0), stop=(c == NKC - 1))
                        nc.vector.tensor_scalar_add(
                            dst[m][:, t4 * 512:(t4 + 1) * 512], pq[:],
                            bs[:, m:m + 1])
                # V natural [128, 256] per 128-T-subchunk
                for i in range(4):
                    kc = 4 * t4 + i
                    pv = mmp.tile([128, 512], F32, tag="mm")
                    for c in range(NKC):
                        nc.tensor.matmul(
                            pv[:, 0:QC],
                            xts[c][:, i * 128:(i + 1) * 128],
                            wv_s[:, c * QC:(c + 1) * QC],
                            start=(c == 0), stop=(c == NKC - 1))
                    vt = vp_s[kc]
                    vt3 = vt[:].rearrange("p (h e) -> p h e", e=2 * D)
                    nc.vector.tensor_tensor(
                        vt3[:, :, 0:D],
                        pv[:, 0:QC].rearrange("p (h d) -> p h d", d=D),
                        bv_b[:].rearrange("p (h d) -> p h d", d=D),
                        op=bass.mybir.AluOpType.add)
                    nc.gpsimd.dma_start(
                        out=vt3[:, :, D:2 * D],
                        in_=on_d.ap().rearrange("p (h d) -> p h d", d=D))

                # attention for q-chunk t4, all heads (S^T layout; O'
                # rows 0:64 = V-accum, rows 64:128 = denominator).
                # S chunks for several kc are packed into one [128, 1024]
                # PSUM tile so one ACTIVATE exps them together.
                lo0, hi0 = t4 * 512, (t4 + 1) * 512
                last_kc = 4 * t4 + 3
                groups = []
                cur, pos = [], 0
                for kc in range(last_kc + 1):
                    lo = max(lo0, kc * 128)
                    n = hi0 - lo
                    npos = pos if pos % 512 + n <= 512 else (pos + 511) // 512 * 512
                    if npos + n > 1024:
                        groups.append(cur)
                        cur, npos = [], 0
                    cur.append((kc, lo, n, npos))
                    pos = npos + n
                groups.append(cur)
                for h in range(HPC):
                    qt_h = qt_s[h // 2][(h % 2) * 64:(h % 2) * 64 + 64, :]
                    kt_h = kt_s[h // 2][(h % 2) * 64:(h % 2) * 64 + 64, :]
                    op_tl = pop.tile([128, 512], F32, tag="po")
                    for grp in groups:
                        sp = ssp.tile([128, 1024], F32, tag="ss")
                        for (kc, lo, n, off) in grp:
                            nc.tensor.matmul(sp[:, off:off + n],
                                             kt_h[:, kc * 128:kc * 128 + 128],
                                             qt_h[:, lo:hi0],
                                             start=True, stop=True)
                        end = grp[-1][3] + grp[-1][2]
                        pt = ppp.tile([128, 1024], F16, tag="p")
                        nc.scalar.activation(pt[:, 0:end], sp[:, 0:end],
                                             AF.Exp, scale=float(SCALE))
                        for (kc, lo, n, off) in grp:
                            if kc * 128 >= lo0:  # diagonal block
                                nc.vector.tensor_mul(
                                    pt[:, off:off + 128],
                                    pt[:, off:off + 128], mask[:])
                            nc.tensor.matmul(
                                op_tl[:, lo - lo0:512],
                                vp_s[kc][:, h * 2 * D:(h + 1) * 2 * D],
                                pt[:, off:off + n],
                                start=(kc == 0), stop=(kc == last_kc))
                    # normalize chunk t4 of head h
                    rc_in = dnp.tile([64, 512], F32, tag="rci", bufs=3)
                    nc.vector.tensor_copy(rc_in[:], op_tl[D:2 * D, :])
                    rc = dnp.tile([64, 512], F32, tag="rc", bufs=3)
                    nc.vector.reciprocal_approx_fast(rc[:], rc_in[:])
                    nc.vector.tensor_mul(
                        ot_s[h // 2][(h % 2) * 64:(h % 2) * 64 + 64, lo0:hi0],
                        op_tl[0:D, :], rc[:])



            # ---- output projection (emitted last: fills the PE during the
            # exp-bound tail of the final attention round) ----
            for n4 in range(NT4):
                lo0, hi0 = n4 * 512, (n4 + 1) * 512
                for m in range(8):
                    py = mmp.tile([128, 512], F32, tag="mm")
                    for k in range(2):
                        nc.tensor.matmul(py[:],
                                         wp_s[k][:, m * 128:(m + 1) * 128],
                                         ot_s[k][:, lo0:hi0],
                                         start=(k == 0), stop=(k == 1))
                    yt_stage = yyp.tile([128, 512], F32, tag="yt", bufs=4)
                    nc.vector.tensor_scalar_add(yt_stage[:], py[:],
                                                bp_s[:, m:m + 1])
                    nc.gpsimd.dma_start(
                        out=yt_d[m * 128:(m + 1) * 128, lo0:hi0],
                        in_=yt_stage[:])

    nc.finalize()
    return nc


def _get_program():
    if "nc" not in _cache:
        import concourse.bass  # noqa: F401  (ensure repo importable early)
        _cache["nc"] = _build()
    return _cache["nc"]


def kernel(x, w_attn, b_attn, w_proj, b_proj):
    x = np.ascontiguousarray(np.asarray(x, dtype=np.float32))
    w_attn = np.ascontiguousarray(np.asarray(w_attn, dtype=np.float32))
    b_attn = np.ascontiguousarray(np.asarray(b_attn, dtype=np.float32))
    w_proj = np.ascontiguousarray(np.asarray(w_proj, dtype=np.float32))
    b_proj = np.ascontiguousarray(np.asarray(b_proj, dtype=np.float32))

    nc = _get_program()
    from concourse.bass_utils import run_bass_kernel_spmd

    mask = np.triu(np.ones((128, 128), dtype=np.float16))
    ident = np.eye(128, dtype=np.float32)
    ones4 = np.ones((128, QC), dtype=np.float16)
    zeros_bp = np.zeros((128, 8), dtype=np.float32)
    bp_full = np.ascontiguousarray(
        b_proj.reshape(8, 128).T.astype(np.float32))

    in_maps = []
    for c in range(NCORES):
        b = c // 4
        hg = c % 4
        q0 = hg * QC
        in_maps.append({
            "x": np.ascontiguousarray(x[b]),
            "wq": np.ascontiguousarray(
                w_attn[:, q0:q0 + QC].astype(np.float16)),
            "wk": np.ascontiguousarray(
                w_attn[:, C + q0:C + q0 + QC].astype(np.float16)),
            "wv": np.ascontiguousarray(
                w_attn[:, 2 * C + q0:2 * C + q0 + QC].astype(np.float16)),
            "bq": np.ascontiguousarray(
                b_attn[q0:q0 + QC].reshape(2, 128).T),
            "bk": np.ascontiguousarray(
                b_attn[C + q0:C + q0 + QC].reshape(2, 128).T),
            "bv": np.ascontiguousarray(
                b_attn[2 * C + q0:2 * C + q0 + QC].reshape(1, QC)),
            "wp": np.ascontiguousarray(
                w_proj[q0:q0 + QC, :].astype(np.float16)),
            "bp": bp_full if hg == 0 else zeros_bp,
            "mask": mask,
            "ident": ident,
            "ones4": ones4,
        })

    trace = bool(os.environ.get("KERNEL_TRACE"))
    res = run_bass_kernel_spmd(nc, in_maps, list(range(NCORES)), trace=trace)
    _cache["last_results"] = res

    out = np.empty((B, T, C), dtype=np.float32)
    for b in range(B):
        acc = res.results[4 * b]["yt"].astype(np.float32)
        for c in range(4 * b + 1, 4 * b + 4):
            acc = acc + res.results[c]["yt"]
        out[b] = acc.T
    return out


# revision 25
# speedup vs baseline: 1.0368x; 1.0368x over previous
"""Causal self-attention (B=2, T=2048, C=1024, H=16, D=64) on 8 trn2 cores.

Sharding: tensor-parallel over (batch, head-group). Core c handles batch
c//4 and heads 4*(c%4) .. 4*(c%4)+4. Each core computes its 4 heads'
QKV projection, causal attention, and the partial output projection
(W_proj row-shard). The 4 partials per batch are summed on the host
(equivalent to the Megatron all-reduce, done at gather time).

On-core dataflow is feature-major ("transposed") throughout:
  X^T (PE transpose) -> Q^T,K^T = W^T X^T ; V natural = X^T^T Wv
  S^T[k,q] = K Q^T per 128-k-chunk (causal: q >= k-chunk start)
  P^T = exp(S^T/8) (ACT), diag-block masked (DVE)
  O'^T[65,q] += V'[k,:]^T P^T  where V' carries a ones column, so row 64
  accumulates the softmax denominator. O^T = O'^T[0:64] * recip(row 64).
  Y^T = W_proj^T O^T + b_proj  -> [1024, 2048] partial per core.

All matmul operands are float32r (fp22 truncation, full PE rate at
free-dim >= 256).
"""
import os
import sys
import numpy as np

B, T, C = 2, 2048, 1024
H, D = 16, 64
HPC = 4                 # heads per core
QC = HPC * D            # 256 qkv cols per core
NCORES = 8
NT = T // 128           # 16 T-chunks of 128
NT4 = T // 512          # 4 T-chunks of 512
NKC = 8                 # contraction chunks over C
SCALE = 1.0 / np.sqrt(D)

_cache = {}


def _build():
    import concourse.bass as bass
    import concourse.bacc as bacc
    import concourse.mybir as mybir
    import concourse.tile as tile

    F32 = mybir.dt.float32
    F32R = mybir.dt.float32r
    F16 = mybir.dt.float16
    AF = mybir.ActivationFunctionType

    nc = bacc.Bacc()
    x_d = nc.dram_tensor("x", [T, C], F16, kind="ExternalInput")
    wq_d = nc.dram_tensor("wq", [C, QC], F16, kind="ExternalInput")
    wk_d = nc.dram_tensor("wk", [C, QC], F16, kind="ExternalInput")
    wv_d = nc.dram_tensor("wv", [C, QC], F16, kind="ExternalInput")
    bq_d = nc.dram_tensor("bq", [128, 2], F32, kind="ExternalInput")
    bk_d = nc.dram_tensor("bk", [128, 2], F32, kind="ExternalInput")
    bv_d = nc.dram_tensor("bv", [1, QC], F32, kind="ExternalInput")
    wp_d = nc.dram_tensor("wp", [QC, C], F16, kind="ExternalInput")
    bp_d = nc.dram_tensor("bp", [128, 8], F32, kind="ExternalInput")
    mask_d = nc.dram_tensor("mask", [128, 128], F16, kind="ExternalInput")
    id_d = nc.dram_tensor("ident", [128, 128], F16, kind="ExternalInput")
    on_d = nc.dram_tensor("ones4", [128, QC], F16, kind="ExternalInput")
    yt_d = nc.dram_tensor("yt", [C, T], F32, kind="ExternalOutput")

    with tile.TileContext(nc) as tc:
        with tc.tile_pool(name="cst", bufs=1) as cst, \
             tc.tile_pool(name="wgt", bufs=1) as wgt, \
             tc.tile_pool(name="xin", bufs=8) as xin, \
             tc.tile_pool(name="xt", bufs=16) as xtp, \
             tc.tile_pool(name="qk", bufs=1) as qkp, \
             tc.tile_pool(name="vv", bufs=1) as vvp, \
             tc.tile_pool(name="pp", bufs=6) as ppp, \
             tc.tile_pool(name="dn", bufs=1) as dnp, \
             tc.tile_pool(name="yy", bufs=2) as yyp, \
             tc.tile_pool(name="mm", bufs=2, space="PSUM") as mmp, \
             tc.tile_pool(name="ss", bufs=2, space="PSUM") as ssp, \
             tc.tile_pool(name="po", bufs=2, space="PSUM") as pop:

            # ---- constants / weights ----
            ident = cst.tile([128, 128], F16, tag="id")
            nc.scalar.dma_start(out=ident[:], in_=id_d[:])
            mask = cst.tile([128, 128], F16, tag="mask")
            nc.scalar.dma_start(out=mask[:], in_=mask_d[:])

            wq_s = wgt.tile([128, NKC * QC], F16, tag="wq")
            wk_s = wgt.tile([128, NKC * QC], F16, tag="wk")
            wv_s = wgt.tile([128, NKC * QC], F16, tag="wv")
            for wd, ws in ((wq_d, wq_s), (wk_d, wk_s), (wv_d, wv_s)):
                nc.scalar.dma_start(
                    out=ws[:].rearrange("p (c n) -> p c n", n=QC),
                    in_=wd.ap().rearrange("(c p) n -> p c n", p=128))
            wp_s = [wgt.tile([128, C], F16, tag=f"wp{k}", name=f"wp{k}") for k in range(2)]
            for k in range(2):
                nc.scalar.dma_start(out=wp_s[k][:],
                                  in_=wp_d[k * 128:(k + 1) * 128, :])
            bq_s = cst.tile([128, 2], F32, tag="bq")
            bk_s = cst.tile([128, 2], F32, tag="bk")
            bp_s = cst.tile([128, 8], F32, tag="bp")
            nc.scalar.dma_start(out=bq_s[:], in_=bq_d[:])
            nc.scalar.dma_start(out=bk_s[:], in_=bk_d[:])
            nc.scalar.dma_start(out=bp_s[:], in_=bp_d[:])
            bv_row = cst.tile([1, QC], F32, tag="bvr")
            nc.scalar.dma_start(out=bv_row[:], in_=bv_d[:])
            bv_b = cst.tile([128, QC], F32, tag="bvb")
            nc.gpsimd.partition_broadcast(bv_b[:], bv_row[:])

            # ---- persistent activations ----
            # Q^T / K^T: two tiles each, heads (2m, 2m+1) stacked on partitions
            qt_s = [qkp.tile([128, T], F16, tag=f"qt{m}", name=f"qt{m}") for m in range(2)]
            kt_s = [qkp.tile([128, T], F16, tag=f"kt{m}", name=f"kt{m}") for m in range(2)]
            # V' tiles: [128, 4*65], per head 64 v-cols + ones col
            vp_s = [vvp.tile([128, HPC * 2 * D], F16, tag=f"vp{i}",
                             name=f"vp{i}") for i in range(NT)]
            # O^T: heads (2m, 2m+1) stacked
            ot_s = [qkp.tile([128, T], F16, tag=f"ot{m}", name=f"ot{m}") for m in range(2)]

            # ---- fused rounds over 512-wide T-chunks ----
            # Round t4: QKV projections for T-chunk t4, then attention
            # q-chunk cq=t4 for all heads (kc = 0..4*t4+3 k-chunks are all
            # available), then the output-projection columns for chunk t4.
            # The Tile scheduler overlaps rounds by dataflow, keeping the
            # PE dense while ACT runs the exps.
            for t4 in range(NT4):
                xt_c = []
                for i in range(4):
                    xl = xin.tile([128, C], F16, tag="x")
                    xt_c.append(xl)
                for half in range(2):
                    for i in range(4):
                        nc.sync.dma_start(
                            out=xt_c[i][:, half * 512:(half + 1) * 512],
                            in_=x_d[(4 * t4 + i) * 128:(4 * t4 + i + 1) * 128,
                                    half * 512:(half + 1) * 512])
                xts = []
                for c in range(NKC):
                    pxt = mmp.tile([128, 512], F16, tag="mm")
                    for i in range(4):
                        nc.tensor.transpose(pxt[:, i * 128:(i + 1) * 128],
                                            xt_c[i][:, c * 128:(c + 1) * 128],
                                            ident[:])
                    xs = xtp.tile([128, 512], F16, tag="xt")
                    nc.vector.tensor_copy(xs[:], pxt[:])
                    xts.append(xs)
                # Q^T, K^T chunks [128, 512]
                for (ws, bs, dst) in ((wq_s, bq_s, qt_s), (wk_s, bk_s, kt_s)):
                    for m in range(2):
                        pq = mmp.tile([128, 512], F32, tag="mm")
                        for c in range(NKC):
                            nc.tensor.matmul(
                                pq[:],
                                ws[:, c * QC + m * 128:c * QC + (m + 1) * 128],
                                xts[c][:],
                                start=(c == 0), stop=(c == NKC - 1))
                        nc.vector.tensor_scalar_add(
                            dst[m][:, t4 * 512:(t4 + 1) * 512], pq[:],
                            bs[:, m:m + 1])
                # V natural [128, 256] per 128-T-subchunk
                for i in range(4):
                    kc = 4 * t4 + i
                    pv = mmp.tile([128, 512], F32, tag="mm")
                    for c in range(NKC):
                        nc.tensor.matmul(
                            pv[:, 0:QC],
                            xts[c][:, i * 128:(i + 1) * 128],
                            wv_s[:, c * QC:(c + 1) * QC],
                            start=(c == 0), stop=(c == NKC - 1))
                    vt = vp_s[kc]
                    vt3 = vt[:].rearrange("p (h e) -> p h e", e=2 * D)
                    nc.vector.tensor_tensor(
                        vt3[:, :, 0:D],
                        pv[:, 0:QC].rearrange("p (h d) -> p h d", d=D),
                        bv_b[:].rearrange("p (h d) -> p h d", d=D),
                        op=bass.mybir.AluOpType.add)
                    nc.gpsimd.dma_start(
                        out=vt3[:, :, D:2 * D],
                        in_=on_d.ap().rearrange("p (h d) -> p h d", d=D))

                # attention for q-chunk t4, all heads (S^T layout; O'
                # rows 0:64 = V-accum, rows 64:128 = denominator).
                # S chunks for several kc are packed into one [128, 1024]
                # PSUM tile so one ACTIVATE exps them together.
                lo0, hi0 = t4 * 512, (t4 + 1) * 512
                last_kc = 4 * t4 + 3
                groups = []
                cur, pos = [], 0
                for kc in range(last_kc + 1):
                    lo = max(lo0, kc * 128)
                    n = hi0 - lo
                    npos = pos if pos % 512 + n <= 512 else (pos + 511) // 512 * 512
                    if npos + n > 1024:
                        groups.append(cur)
                        cur, npos = [], 0
                    cur.append((kc, lo, n, npos))
                    pos = npos + n
                groups.append(cur)
                for h in range(HPC):
                    qt_h = qt_s[h // 2][(h % 2) * 64:(h % 2) * 64 + 64, :]
                    kt_h = kt_s[h // 2][(h % 2) * 64:(h % 2) * 64 + 64, :]
                    op_tl = pop.tile([128, 512], F32, tag="po")
                    for grp in groups:
                        sp = ssp.tile([128, 1024], F32, tag="ss")
                        for (kc, lo, n, off) in grp:
                            nc.tensor.matmul(sp[:, off:off + n],
                                             kt_h[:, kc * 128:kc * 128 + 128],
                                             qt_h[:, lo:hi0],
                                             start=True, stop=True)
                        end = grp[-1][3] + grp[-1][2]
                        pt = ppp.tile([128, 1024], F16, tag="p")
                        nc.scalar.activation(pt[:, 0:end], sp[:, 0:end],
                                             AF.Exp, scale=float(SCALE))
                        for (kc, lo, n, off) in grp:
                            if kc * 128 >= lo0:  # diagonal block
                                nc.vector.tensor_mul(
                                    pt[:, off:off + 128],
                                    pt[:, off:off + 128], mask[:])
                            nc.tensor.matmul(
                                op_tl[:, lo - lo0:512],
                                vp_s[kc][:, h * 2 * D:(h + 1) * 2 * D],
                                pt[:, off:off + n],
                                start=(kc == 0), stop=(kc == last_kc))
                    # normalize chunk t4 of head h
                    rc_in = dnp.tile([64, 512], F32, tag="rci", bufs=3)
                    nc.vector.tensor_copy(rc_in[:], op_tl[D:2 * D, :])
                    rc = dnp.tile([64, 512], F32, tag="rc", bufs=3)
                    nc.vector.reciprocal_approx_fast(rc[:], rc_in[:])
                    nc.vector.tensor_mul(
                        ot_s[h // 2][(h % 2) * 64:(h % 2) * 64 + 64, lo0:hi0],
                        op_tl[0:D, :], rc[:])



            # ---- output projection (emitted last: fills the PE during the
            # exp-bound tail of the final attention round) ----
            for n4 in range(NT4):
                lo0, hi0 = n4 * 512, (n4 + 1) * 512
                for m in range(8):
                    py = mmp.tile([128, 512], F32, tag="mm")
                    for k in range(2):
                        nc.tensor.matmul(py[:],
                                         wp_s[k][:, m * 128:(m + 1) * 128],
                                         ot_s[k][:, lo0:hi0],
                                         start=(k == 0), stop=(k == 1))
                    yt_stage = yyp.tile([128, 512], F32, tag="yt", bufs=4)
                    nc.vector.tensor_scalar_add(yt_stage[:], py[:],
                                                bp_s[:, m:m + 1])
                    nc.gpsimd.dma_start(
                        out=yt_d[m * 128:(m + 1) * 128, lo0:hi0],
                        in_=yt_stage[:])

    nc.finalize()
    return nc


def _get_program():
    if "nc" not in _cache:
        import concourse.bass  # noqa: F401  (ensure repo importable early)
        _cache["nc"] = _build()
    return _cache["nc"]


def kernel(x, w_attn, b_attn, w_proj, b_proj):
    x = np.ascontiguousarray(np.asarray(x, dtype=np.float32))
    w_attn = np.ascontiguousarray(np.asarray(w_attn, dtype=np.float32))
    b_attn = np.ascontiguousarray(np.asarray(b_attn, dtype=np.float32))
    w_proj = np.ascontiguousarray(np.asarray(w_proj, dtype=np.float32))
    b_proj = np.ascontiguousarray(np.asarray(b_proj, dtype=np.float32))

    nc = _get_program()
    from concourse.bass_utils import run_bass_kernel_spmd

    mask = np.triu(np.ones((128, 128), dtype=np.float16))
    ident = np.eye(128, dtype=np.float16)
    ones4 = np.ones((128, QC), dtype=np.float16)
    zeros_bp = np.zeros((128, 8), dtype=np.float32)
    bp_full = np.ascontiguousarray(
        b_proj.reshape(8, 128).T.astype(np.float32))

    in_maps = []
    for c in range(NCORES):
        b = c // 4
        hg = c % 4
        q0 = hg * QC
        in_maps.append({
            "x": np.ascontiguousarray(x[b].astype(np.float16)),
            "wq": np.ascontiguousarray(
                w_attn[:, q0:q0 + QC].astype(np.float16)),
            "wk": np.ascontiguousarray(
                w_attn[:, C + q0:C + q0 + QC].astype(np.float16)),
            "wv": np.ascontiguousarray(
                w_attn[:, 2 * C + q0:2 * C + q0 + QC].astype(np.float16)),
            "bq": np.ascontiguousarray(
                b_attn[q0:q0 + QC].reshape(2, 128).T),
            "bk": np.ascontiguousarray(
                b_attn[C + q0:C + q0 + QC].reshape(2, 128).T),
            "bv": np.ascontiguousarray(
                b_attn[2 * C + q0:2 * C + q0 + QC].reshape(1, QC)),
            "wp": np.ascontiguousarray(
                w_proj[q0:q0 + QC, :].astype(np.float16)),
            "bp": bp_full if hg == 0 else zeros_bp,
            "mask": mask,
            "ident": ident,
            "ones4": ones4,
        })

    trace = bool(os.environ.get("KERNEL_TRACE"))
    res = run_bass_kernel_spmd(nc, in_maps, list(range(NCORES)), trace=trace)
    _cache["last_results"] = res

    out = np.empty((B, T, C), dtype=np.float32)
    for b in range(B):
        acc = res.results[4 * b]["yt"].astype(np.float32)
        for c in range(4 * b + 1, 4 * b + 4):
            acc = acc + res.results[c]["yt"]
        out[b] = acc.T
    return out
